# revision 35
# baseline (speedup 1.0000x reference)
"""Multi-head attention Trainium2 kernel (B=4, T=2048, C=1024, H=16, D=64).

Sharding: 8 cores = 4 batches x 2 head-groups (data parallel on B, tensor
parallel on H). Each core computes attention for 1 batch and 8 heads plus the
partial out-projection for its head rows; the host sums the two partials per
batch (the out-proj "all-reduce") and adds the bias.

Device layout notes (per core):
  xT  [C, T]   bf16  x[b] transposed + repacked on host into query-column
               quarters so each quarter is one contiguous ~1MB DMA
  wq/wk/wv [C, 512] bf16 per-head-group column slices of w_qkv, repacked
               pair-major (wk/wq) / ctile-major (wv) for single-DMA loads
  wo  [512, C] bf16  row slice of w_out, pair-major
  y   [T, C]   bf16  partial output (host sums the two hg partials in f32)

  Inputs stream in first-use order (wk, wv, wq pair-0, xT quarters; wq
  pairs 1-3 + wo trail after the upfront chains); the upfront K0/Q0
  projection chunks and the first four V chunks chase the quarter arrival
  so the PE has work during the DMA-bound window. The final out-proj row
  groups are pre-accumulated (pairs 0-2) while the last softmax
  normalization chain runs, so the tail only adds the pair-3 rows.

  QT/KT: [D,T] per head, two heads packed per 128-partition tile. Scores
  S^T[k,q] matmuls alternate the two heads (disjoint PE row groups) so
  consecutive matmuls can overlap in the array. The exp of the scores is
  split between ScalarE (exact exp, scaled by K) and a custom 8-stage DVE
  ucode op computing K*exp(x/8) ~= (c1*(x+c0)^2+c2)^16 (error <0.2% where
  softmax mass lives); the constant K cancels in the softmax ratio. expS
  lands in an interleaved bf16 ring in SBUF. V is kept natural [T,D] with a
  appended ones column so the M=65 PV matmul produces O^T (rows 0..63) and
  the softmax denominators (row 64) in one pass. Denominators are copied to
  partition 0 by ScalarE, reciprocal via fast DVE approx, partition
  broadcast on GpSimd, normalization mul on DVE, then the
  out-projection consumes Theta^T as the stationary operand; y is copied to
  bf16 by ScalarE and DMA'd out.
"""

import numpy as np
import ml_dtypes

import concourse.bacc as bacc
import concourse.mybir as mybir
import concourse.tile as tile
from concourse.bass_utils import run_bass_kernel_spmd

B, T, C, H, D = 4, 2048, 1024, 16, 64
HPC = 8          # heads per core
PAIRS = HPC // 2
CT = C // 128    # 8 contraction tiles for projections
TT = T // 128    # 16 t-tiles (also k-tiles of attention)
QC = T // 512    # 4 query chunks
JC = C // 512    # 2 out-proj column chunks
BF16 = mybir.dt.bfloat16
F32 = mybir.dt.float32
EXP = mybir.ActivationFunctionType.Exp

_CACHED_NC = None

# ---- custom DVE exp op: K*exp(s/8) ~= (C1*(s+C0)^2 + C2)^16 --------------
# Constants fitted to minimize attention-output error for logits ~ N(0,1)
# mixed half/half with the exact-exp (ScalarE) path; K = 0.20595367 is the
# shared scale (cancels in softmax), applied on the ACT side via bias=ln K.
EXP_C0 = 113.8532448      # 8*a: input is raw scores s, logit = s/8
EXP_C1 = 3.104248719e-05  # c/64
EXP_C2 = 0.504467297
EXP_ACT_BIAS = -1.5801040383996299  # ln(0.20595367)


def _register_exp_op():
    from concourse.dve_ops import (DveOp, OPS, CUSTOM_DVE_SPECS,
                                   _SUB_OPCODE_FOR_NAME)
    from concourse.dve_spec import Spec, Src0, C0, C1, C2, lower, _has_src1
    from concourse.dve_uop import DveOpSpec

    name = "EXP_POW16_ANT"
    if name in _SUB_OPCODE_FOR_NAME:
        return next(o for o in OPS if o.name == name)
    _t = Src0 + C0
    _s = _t * _t
    _u = _s * C1
    _h = _u + C2
    _h2 = _h * _h
    _h4 = _h2 * _h2
    _h8 = _h4 * _h4
    body = _h8 * _h8

    def _ref(in0, in1, c0, c1, c2):
        h = (c1 * (in0 + c0).astype(np.float32) ** 2 + c2).astype(np.float32)
        return h ** 16

    spec = Spec(body=body, reference=_ref)
    row = 17
    _SUB_OPCODE_FOR_NAME[name] = row
    tmp = DveOpSpec(name=name, opcode=row, uops=lower(spec, ver="v3"),
                    rd1_en=_has_src1(spec))
    op = DveOp(name, spec, subdim=False,
               uops_sha={"v3": tmp.sha("v3"), "v4": "unpinned"})
    OPS.append(op)
    CUSTOM_DVE_SPECS[name] = spec
    return op


EXP_OP = _register_exp_op()

# kt tiles whose exp runs on the DVE custom op (rest go to ScalarE), strict
# even/odd alternation so the two engines drain the score psum banks in
# lockstep and adjacent score pairs can stay clumped (row-tile concurrency).
DVE_KT = frozenset((1, 3, 5, 7, 9, 11, 13, 15))


def _emit(nc, tc, xT_d, wk_d, wq0_d, wqr_d, wv_d, wo_d, y_d):
    import contextlib
    with contextlib.ExitStack() as ctx:
        persist = ctx.enter_context(tc.tile_pool(name="persist", bufs=1))
        work = ctx.enter_context(tc.tile_pool(name="work", bufs=2))
        spsum = ctx.enter_context(tc.tile_pool(name="spsum", bufs=2, space="PSUM"))
        apsum = ctx.enter_context(tc.tile_pool(name="apsum", bufs=2, space="PSUM"))
        ppsum = ctx.enter_context(tc.tile_pool(name="ppsum", bufs=2, space="PSUM"))

        # ---- static loads ----
        # Consolidated input DMAs: one large contiguous transfer per logical
        # input (host pre-packs so both DRAM and SBUF sides are contiguous =
        # 2-8KB per-partition lines; >=1MB transfers run near peak BW).
        # Need-order: wk + wq pair-0 slice first, then xT in query-column
        # quarters (the upfront K0/Q0 chunks chase quarter arrival), wv before
        # the V weave; wq pairs 1-3 and wo trigger after the upfront chains.
        # SBUF layouts:
        #   xT_all [128, (qc, i, 512)]   wk/wq_all [128, (pair, i, 128)]
        #   wv_all [128, (i, 512)]       wo_all [128, (pair, 1024)]
        xT_all = persist.tile([128, 4 * 4096], BF16, tag="xTall", name="xTall")
        wk_all = persist.tile([128, 4096], BF16, tag="wkall", name="wkall")
        wq_all = persist.tile([128, 4096], BF16, tag="wqall", name="wqall")
        wv_all = persist.tile([128, 4096], BF16, tag="wvall", name="wvall")
        wo_all = persist.tile([128, 4096], BF16, tag="woall", name="woall")

        nc.sync.dma_start(out=wk_all, in_=wk_d[:, :])
        nc.scalar.dma_start(out=wq_all[:, 0:1024], in_=wq0_d[:, :])
        nc.scalar.dma_start(out=wv_all, in_=wv_d[:, :])
        for qc in range(QC):
            nc.sync.dma_start(out=xT_all[:, qc * 4096:(qc + 1) * 4096],
                              in_=xT_d[qc * 128:(qc + 1) * 128, :])

        def xq(i, qc):
            # xT ctile i, query-column chunk qc  [128, 512]
            o = qc * 4096 + i * 512
            return xT_all[:, o:o + 512]

        def xv(i, tt):
            # xT ctile i, t-chunk tt columns  [128, 128]
            o = (tt // 4) * 4096 + i * 512 + (tt % 4) * 128
            return xT_all[:, o:o + 128]

        def wslice(wname, c, p):
            # w ctile c, head-pair p column slice  [128, 128]
            all_ = wk_all if wname == "wk" else wq_all
            return all_[:, p * 1024 + c * 128:p * 1024 + (c + 1) * 128]

        ebias = persist.tile([128, 1], F32, tag="ebias", name="ebias")
        nc.vector.memset(ebias, EXP_ACT_BIAS)

        # V natural [T, 512] + appended ones column per head:
        # vaug [128, 8, 65] with [:, h, 0:64] = V_h and [:, h, 64] = 1
        vaug = [persist.tile([128, HPC, D + 1], BF16, tag=f"vaug{tt}", name=f"vaug{tt}")
                for tt in range(TT)]

        def v_chunk(tt):
            vt = vaug[tt]
            pool, tg = (ppsum, "proj") if tt % 2 == 0 else (apsum, "acc")
            ps = pool.tile([128, 512], F32, tag=tg, name="vps")
            for c in range(CT):
                nc.tensor.matmul(ps, lhsT=xv(c, tt),
                                 rhs=wv_all[:, c * 512:(c + 1) * 512],
                                 start=(c == 0), stop=(c == CT - 1))
            nc.vector.tensor_copy(
                out=vt[:, :, 0:D],
                in_=ps.rearrange("p (h d) -> p h d", h=HPC))
            nc.vector.memset(vt[:, :, D:D + 1], 1.0)

        # Q^T / K^T tiles [128 = 2 heads x 64, T]; filled lazily per pair so
        # later pairs' projections overlap earlier pairs' attention
        qt_sb = [persist.tile([128, T], BF16, tag="qt", bufs=2, name=f"qt{p}")
                 for p in range(PAIRS)]
        kt_sb = [persist.tile([128, T], BF16, tag="kt", bufs=2, name=f"kt{p}")
                 for p in range(PAIRS)]

        def project_chunk(p, dst, wname, qc):
            ps = ppsum.tile([128, 512], F32, tag="proj", name="qkps")
            for c in range(CT):
                nc.tensor.matmul(
                    ps,
                    lhsT=wslice(wname, c, p),
                    rhs=xq(c, qc),
                    start=(c == 0), stop=(c == CT - 1))
            # evacuate on ACT ([128,512] copy is cheaper there and DVE is
            # loaded with the exp split)
            nc.scalar.copy(out=dst[:, qc * 512:(qc + 1) * 512], in_=ps)

        # pair-0 K/Q upfront, interleaved per query chunk so the chains chase
        # the qc-major xT quarter arrival (the DMA-wait window is free PE
        # time); scores (0,0) need all of K0 so sections start right after.
        project_chunk(0, kt_sb[0], "wk", 0)
        project_chunk(0, qt_sb[0], "wq", 0)
        # V chunks 0-3 need only wv + the first xT quarter: they fill the
        # PE idle while the remaining quarters stream in
        for tt in range(4):
            v_chunk(tt)
        for qc in range(1, QC):
            project_chunk(0, kt_sb[0], "wk", qc)
            project_chunk(0, qt_sb[0], "wq", qc)
        # non-critical weights trail the xT quarters in the scalar queue
        nc.scalar.dma_start(out=wq_all[:, 1024:4096], in_=wqr_d[:, :])
        nc.scalar.dma_start(out=wo_all, in_=wo_d[:, :])

        # ---- attention ----
        # expS ring: interleaved [h0 kt | h1 kt] units of 512, RING units so
        # exp of section s+1 can run ahead while PV of section s drains.
        RING = 56
        exps = persist.tile([128, RING * 512], BF16, tag="expS", name="expS")
        tht_sb = [persist.tile([128, T], BF16, tag=f"tht{p}", name=f"tht{p}")
                  for p in range(PAIRS)]
        # just-in-time projection fillers emitted after each (p, qc) section
        fillers = {
            (0, 1): [(1, "wk", 0), (1, "wk", 1), (1, "wk", 2)],
            (0, 2): [(1, "wk", 3), (1, "wq", 0), (1, "wq", 1)],
            (0, 3): [(1, "wq", 2), (1, "wq", 3)],
            (1, 0): [(2, "wk", 0), (2, "wk", 1)],
            (1, 1): [(2, "wk", 2), (2, "wk", 3)],
            (1, 2): [(2, "wq", 0), (2, "wq", 1)],
            (1, 3): [(2, "wq", 2), (2, "wq", 3)],
            (2, 0): [(3, "wk", 0), (3, "wk", 1)],
            (2, 1): [(3, "wk", 2), (3, "wk", 3)],
            (2, 2): [(3, "wq", 0), (3, "wq", 1)],
            (2, 3): [(3, "wq", 2), (3, "wq", 3)],
        }

        def out_proj_group(tt):
            # per-jc copy + DMA so the first half drains while the second
            # half's accumulation is still running
            ysb = work.tile([128, C], BF16, tag="ysb", bufs=3, name="ysb")
            for jc in range(JC):
                jsl = slice(jc * 512, (jc + 1) * 512)
                # both halves on the ppsum ring: keeps the apsum ring free
                # for the PV accumulators, so PV start never waits on a
                # group evacuation copy
                yps = ppsum.tile([128, 512], F32, tag="proj", name="yps")
                for pp in range(PAIRS):
                    nc.tensor.matmul(
                        yps, lhsT=tht_sb[pp][:, tt * 128:(tt + 1) * 128],
                        rhs=wo_all[:, pp * 1024 + jc * 512:
                                   pp * 1024 + (jc + 1) * 512],
                        start=(pp == 0), stop=(pp == PAIRS - 1))
                # evacuation split across ScalarE/VectorE so neither paces
                # the out-proj chains
                if jc == 0:
                    nc.scalar.copy(out=ysb[:, jsl], in_=yps)
                else:
                    nc.vector.tensor_copy(out=ysb[:, jsl], in_=yps)
                eng = nc.sync if (tt + jc) % 2 == 0 else nc.gpsimd
                eng.dma_start(out=y_d[tt * 128:(tt + 1) * 128, jsl],
                              in_=ysb[:, jsl])

        ring_base = 0
        for p in range(PAIRS):
            for qc in range(QC):
                qsl = slice(qc * 512, (qc + 1) * 512)

                def unit(kt, lh):
                    u = (ring_base + 2 * kt + lh) % RING
                    return slice(u * 512, (u + 1) * 512)

                # scores + exp: adjacent matmuls alternate PE row groups
                # (h0 rows 0-63, h1 rows 64-127) so they can overlap
                for kt in range(TT):
                    ps = spsum.tile([128, 1024], F32, tag="mm", name="sps")
                    with tc.high_priority():
                        for lh in range(2):
                            hsl = slice(lh * 64, (lh + 1) * 64)
                            nc.tensor.matmul(
                                ps[:, lh * 512:(lh + 1) * 512],
                                lhsT=kt_sb[p][hsl, kt * 128:(kt + 1) * 128],
                                rhs=qt_sb[p][hsl, qsl],
                                start=True, stop=True)
                    # exp split across BOTH engines per kt (one head-bank
                    # each, roles alternating by kt so every head keeps the
                    # 50/50 exact/approx mix): halves the pair's exp latency
                    # so the score banks recycle sooner
                    dve_lh = kt % 2
                    for lh in range(2):
                        u = (ring_base + 2 * kt + lh) % RING
                        eout = exps[:, u * 512:(u + 1) * 512]
                        pss = ps[:, lh * 512:(lh + 1) * 512]
                        if lh == dve_lh:
                            nc.vector._custom_dve(EXP_OP, out=eout, in0=pss,
                                                  s0=EXP_C0, s1=EXP_C1,
                                                  imm2=EXP_C2)
                        else:
                            nc.scalar.activation(out=eout, in_=pss, func=EXP,
                                                 scale=0.125, bias=ebias)
                    if p == 0 and qc == 0 and kt >= 4:
                        # V projection woven into the exp-paced score loop
                        # (chunks 0-3 were projected in the prologue)
                        v_chunk(kt)
                # out-projection of the previous qc chunk, placed between
                # scores and PV
                if p == PAIRS - 1 and qc >= 1:
                    for tt in range(4 * (qc - 1), 4 * qc):
                        out_proj_group(tt)

                # PV: both heads' accumulation chains interleaved; row 0 of
                # each chain is the softmax denominator (ones-first vaug)
                ops = [apsum.tile([D + 1, 512], F32, tag="acc", name=f"ops{lh}")
                       for lh in range(2)]
                for kt in range(TT):
                    for lh in range(2):
                        nc.tensor.matmul(
                            ops[lh], lhsT=vaug[kt][:, 2 * p + lh, :],
                            rhs=exps[:, unit(kt, lh)],
                            start=(kt == 0), stop=(kt == TT - 1))
                for lh in range(2):
                    # move the sums to partition 0 on ACT (the custom-DVE
                    # fast reciprocal misreads partition-shifted inputs, and
                    # PSUM partition starts must be 32-aligned anyway)
                    ssb = work.tile([1, 512], F32, tag="ssb", name="ssb")
                    nc.scalar.copy(out=ssb, in_=ops[lh][D:D + 1, :])
                    rsb = work.tile([1, 512], F32, tag="rsb", name="rsb")
                    nc.vector.reciprocal_approx_fast(out=rsb, in_=ssb)
                    rbc = work.tile([64, 512], F32, tag="rbc", name="rbc")
                    nc.gpsimd.partition_broadcast(rbc, rsb)
                    nc.vector.tensor_mul(
                        out=tht_sb[p][lh * 64:(lh + 1) * 64, qsl],
                        in0=ops[lh][0:D, :], in1=rbc)
                ring_base = (ring_base + 2 * TT) % RING
                for fp, wname, fqc in fillers.get((p, qc), []):
                    dst = qt_sb[fp] if wname == "wq" else kt_sb[fp]
                    project_chunk(fp, dst, wname, fqc)

        # ---- tail ----
        # Pre-accumulate pairs 0-2 of out-proj groups 12-14 into borrowed
        # PSUM (scores banks are drained, ppsum has no fillers left). These
        # MMs depend only on earlier sections' theta, so they keep the PE
        # streaming (and HAM warm) while the final normalization chain runs;
        # then each group finishes with just the pair-3 rows. tt15 reuses the
        # spsum ring slot freed by tt12's evacuation.
        pre_yps = {}
        for tt in (12, 13):
            pt = spsum.tile([128, 1024], F32, tag="mm", name=f"pre{tt}")
            pre_yps[tt] = (pt[:, 0:512], pt[:, 512:1024])
        pre_yps[14] = (
            ppsum.tile([128, 512], F32, tag="proj", name="pre14a"),
            ppsum.tile([128, 512], F32, tag="proj", name="pre14b"))
        for tt in (12, 13, 14):
            for jc in range(JC):
                for pp in range(PAIRS - 1):
                    nc.tensor.matmul(
                        pre_yps[tt][jc],
                        lhsT=tht_sb[pp][:, tt * 128:(tt + 1) * 128],
                        rhs=wo_all[:, pp * 1024 + jc * 512:
                                   pp * 1024 + (jc + 1) * 512],
                        start=(pp == 0), stop=False)
        for tt in (12, 13, 14):
            ysb = work.tile([128, C], BF16, tag="ysb", bufs=3, name="ysb")
            for jc in range(JC):
                nc.tensor.matmul(
                    pre_yps[tt][jc],
                    lhsT=tht_sb[3][:, tt * 128:(tt + 1) * 128],
                    rhs=wo_all[:, 3 * 1024 + jc * 512:3 * 1024 + (jc + 1) * 512],
                    start=False, stop=True)
            for jc in range(JC):
                jsl = slice(jc * 512, (jc + 1) * 512)
                if jc == 0:
                    nc.scalar.copy(out=ysb[:, jsl], in_=pre_yps[tt][jc])
                else:
                    nc.vector.tensor_copy(out=ysb[:, jsl], in_=pre_yps[tt][jc])
                eng = nc.sync if (tt + jc) % 2 == 0 else nc.gpsimd
                eng.dma_start(out=y_d[tt * 128:(tt + 1) * 128, jsl],
                              in_=ysb[:, jsl])
        # tt15 runs through the apsum ring: its slots free as soon as the
        # final normalization reads the PV accumulators (~norm time), unlike
        # the spsum slots which would wait for the pre12 evacuation copies
        ysb = work.tile([128, C], BF16, tag="ysb", bufs=3, name="ysb")
        for jc in range(JC):
            jsl = slice(jc * 512, (jc + 1) * 512)
            yps = apsum.tile([128, 512], F32, tag="acc", name="yps15")
            for pp in range(PAIRS):
                nc.tensor.matmul(
                    yps,
                    lhsT=tht_sb[pp][:, 15 * 128:16 * 128],
                    rhs=wo_all[:, pp * 1024 + jc * 512:
                               pp * 1024 + (jc + 1) * 512],
                    start=(pp == 0), stop=(pp == PAIRS - 1))
            if jc == 0:
                nc.scalar.copy(out=ysb[:, jsl], in_=yps)
            else:
                nc.vector.tensor_copy(out=ysb[:, jsl], in_=yps)
            eng = nc.sync if jc == 0 else nc.gpsimd
            eng.dma_start(out=y_d[15 * 128:16 * 128, jsl], in_=ysb[:, jsl])


def _build():
    nc = bacc.Bacc("TRN2", target_bir_lowering=False)
    # host-prepacked layouts (see make_in_maps): xT quarters [qc][p, (i, c)],
    # wk/wq pair-major [p, (pair, i, 128)], wv [p, (i, 512)], wo [p, (pair, C)]
    xT_d = nc.dram_tensor("xT", [512, 4096], BF16, kind="ExternalInput")
    wk_d = nc.dram_tensor("wk", [128, 4096], BF16, kind="ExternalInput")
    wq0_d = nc.dram_tensor("wq0", [128, 1024], BF16, kind="ExternalInput")
    wqr_d = nc.dram_tensor("wqr", [128, 3072], BF16, kind="ExternalInput")
    wv_d = nc.dram_tensor("wv", [128, 4096], BF16, kind="ExternalInput")
    wo_d = nc.dram_tensor("wo", [128, 4096], BF16, kind="ExternalInput")
    y_d = nc.dram_tensor("y", [T, C], BF16, kind="ExternalOutput")
    with tile.TileContext(nc) as tc:
        _emit(nc, tc, xT_d, wk_d, wq0_d, wqr_d, wv_d, wo_d, y_d)
    if not nc.is_finalized():
        nc.finalize()
    return nc


def get_nc():
    global _CACHED_NC
    if _CACHED_NC is None:
        _CACHED_NC = _build()
    return _CACHED_NC


def make_in_maps(x, w_qkv, w_out, b_out):
    bf = ml_dtypes.bfloat16
    x = np.asarray(x, dtype=np.float32)
    w_qkv = np.asarray(w_qkv, dtype=np.float32)
    w_out = np.asarray(w_out, dtype=np.float32)
    in_maps = []
    for core in range(8):
        b, hg = core // 2, core % 2
        cs = slice(hg * 512, (hg + 1) * 512)
        xT = x[b].T  # [C, T]
        wq = w_qkv[:, 0 * C:][:, cs]  # [C, 512]
        wk = w_qkv[:, 1 * C:][:, cs]
        wv = w_qkv[:, 2 * C:][:, cs]
        wo = w_out[cs, :]  # [512, C]
        # pack so each DMA is contiguous on both DRAM and SBUF sides:
        #   xT   -> [qc, p, (i, 512)]    (query-column quarters)
        #   wk   -> [p, (pair, i, 128)]  (head-pair-major column slices)
        #   wq   -> pair-0 block [p, (i, 128)] + pairs 1-3 [p, (pair, i, 128)]
        #   wv   -> [p, (i, 512)]
        #   wo   -> [p, (pair, C)]
        xT4 = xT.reshape(8, 128, 4, 512).transpose(2, 1, 0, 3).reshape(512, 4096)
        wk4 = wk.reshape(8, 128, 4, 128).transpose(1, 2, 0, 3).reshape(128, 4096)
        wq4 = wq.reshape(8, 128, 4, 128)
        wq0 = wq4[:, :, 0, :].transpose(1, 0, 2).reshape(128, 1024)
        wqr = wq4[:, :, 1:, :].transpose(1, 2, 0, 3).reshape(128, 3072)
        wv4 = wv.reshape(8, 128, 512).transpose(1, 0, 2).reshape(128, 4096)
        wo4 = wo.reshape(4, 128, 1024).transpose(1, 0, 2).reshape(128, 4096)
        in_maps.append({
            "xT": np.ascontiguousarray(xT4).astype(bf),
            "wk": np.ascontiguousarray(wk4).astype(bf),
            "wq0": np.ascontiguousarray(wq0).astype(bf),
            "wqr": np.ascontiguousarray(wqr).astype(bf),
            "wv": np.ascontiguousarray(wv4).astype(bf),
            "wo": np.ascontiguousarray(wo4).astype(bf),
        })
    return in_maps


def _ensure_ntff_hook():
    """Register the axon NTFF profile hook if the container's antenv lacks
    axon_hooks (test/profiling use only; never needed for plain kernel())."""
    import sys
    import types
    try:
        from antenv import axon_hooks  # noqa: F401
    except ImportError:
        mod = types.ModuleType("antenv.axon_hooks")
        mod._hook = None

        def set_axon_ntff_profile_hook(hook, _m=mod):
            _m._hook = hook

        def get_axon_ntff_profile_hook(_m=mod):
            return _m._hook

        mod.set_axon_ntff_profile_hook = set_axon_ntff_profile_hook
        mod.get_axon_ntff_profile_hook = get_axon_ntff_profile_hook
        sys.modules["antenv.axon_hooks"] = mod
        import antenv
        antenv.axon_hooks = mod
    import antenv.axon_hooks as ah
    if ah.get_axon_ntff_profile_hook() is None:
        from trn_agent_boot.trn_boot import _ntff_profile_via_ctypes
        ah.set_axon_ntff_profile_hook(
            _ntff_profile_via_ctypes("/opt/axon/libaxon_pjrt.so"))


def kernel(x, w_qkv, w_out, b_out, _trace=False, _trace_kwargs=None):
    nc = get_nc()
    in_maps = make_in_maps(x, w_qkv, w_out, b_out)
    kwargs = {}
    if _trace:
        try:
            _ensure_ntff_hook()
        except Exception as e:
            print(f"NTFF hook setup failed ({e}); running without trace")
        else:
            kwargs.update(trace=True, **(_trace_kwargs or {}))
    res = run_bass_kernel_spmd(nc, in_maps, core_ids=list(range(8)), **kwargs)
    bias = np.asarray(b_out, dtype=np.float32)
    out = np.empty((B, T, C), dtype=np.float32)
    for b in range(B):
        out[b] = (res.results[2 * b]["y"].astype(np.float32)
                  + res.results[2 * b + 1]["y"].astype(np.float32) + bias)
    if _trace:
        return out, res
    return out



# revision 36
# speedup vs baseline: 1.1203x; 1.1203x over previous
"""Multi-head attention Trainium2 kernel (B=4, T=2048, C=1024, H=16, D=64).

Sharding: 8 cores = 4 batches x 2 head-groups (data parallel on B, tensor
parallel on H). Each core computes attention for 1 batch and 8 heads plus the
partial out-projection for its head rows; the host sums the two partials per
batch (the out-proj "all-reduce") and adds the bias.

Device layout notes (per core):
  xT  [C, T]   bf16  x[b] transposed + repacked on host into query-column
               quarters so each quarter is one contiguous ~1MB DMA
  wq/wk/wv [C, 512] bf16 per-head-group column slices of w_qkv, repacked
               pair-major (wk/wq) / ctile-major (wv) for single-DMA loads
  wo  [512, C] bf16  row slice of w_out, pair-major
  y   [T, C]   bf16  partial output (host sums the two hg partials in f32)

  Inputs stream in first-use order (wk, wv, wq pair-0, xT quarters; wq
  pairs 1-3 + wo trail after the upfront chains); the upfront K0/Q0
  projection chunks and the first four V chunks chase the quarter arrival
  so the PE has work during the DMA-bound window. The final out-proj row
  groups are pre-accumulated (pairs 0-2) while the last softmax
  normalization chain runs, so the tail only adds the pair-3 rows.

  QT/KT: [D,T] per head, two heads packed per 128-partition tile. Scores
  S^T[k,q] matmuls alternate the two heads (disjoint PE row groups) so
  consecutive matmuls can overlap in the array. The exp of the scores is
  split between ScalarE (exact exp, scaled by K) and a custom 8-stage DVE
  ucode op computing K*exp(x/8) ~= (c1*(x+c0)^2+c2)^16 (error <0.2% where
  softmax mass lives); the constant K cancels in the softmax ratio. expS
  lands in an interleaved bf16 ring in SBUF. V is kept natural [T,D] with a
  appended ones column so the M=65 PV matmul produces O^T (rows 0..63) and
  the softmax denominators (row 64) in one pass. Denominators are copied to
  partition 0 by ScalarE, reciprocal via fast DVE approx, partition
  broadcast on GpSimd, normalization mul on DVE, then the
  out-projection consumes Theta^T as the stationary operand; y is copied to
  bf16 by ScalarE and DMA'd out.
"""

import numpy as np
import ml_dtypes

import concourse.bacc as bacc
import concourse.mybir as mybir
import concourse.tile as tile
from concourse.bass_utils import run_bass_kernel_spmd

B, T, C, H, D = 4, 2048, 1024, 16, 64
HPC = 8          # heads per core
PAIRS = HPC // 2
CT = C // 128    # 8 contraction tiles for projections
TT = T // 128    # 16 t-tiles (also k-tiles of attention)
QC = T // 512    # 4 query chunks
JC = C // 512    # 2 out-proj column chunks
BF16 = mybir.dt.bfloat16
F32 = mybir.dt.float32
EXP = mybir.ActivationFunctionType.Exp

_CACHED_NC = None

# ---- custom DVE exp op: K*exp(s/8) ~= (C1*(s+C0)^2 + C2)^16 --------------
# Constants fitted to minimize attention-output error for logits ~ N(0,1)
# mixed half/half with the exact-exp (ScalarE) path; K = 0.20595367 is the
# shared scale (cancels in softmax), applied on the ACT side via bias=ln K.
EXP_C0 = 113.8532448      # 8*a: input is raw scores s, logit = s/8
EXP_C1 = 3.104248719e-05  # c/64
EXP_C2 = 0.504467297
EXP_ACT_BIAS = -1.5801040383996299  # ln(0.20595367)


def _register_exp_op():
    from concourse.dve_ops import (DveOp, OPS, CUSTOM_DVE_SPECS,
                                   _SUB_OPCODE_FOR_NAME)
    from concourse.dve_spec import Spec, Src0, C0, C1, C2, lower, _has_src1
    from concourse.dve_uop import DveOpSpec

    name = "EXP_POW16_ANT"
    if name in _SUB_OPCODE_FOR_NAME:
        return next(o for o in OPS if o.name == name)
    _t = Src0 + C0
    _s = _t * _t
    _u = _s * C1
    _h = _u + C2
    _h2 = _h * _h
    _h4 = _h2 * _h2
    _h8 = _h4 * _h4
    body = _h8 * _h8

    def _ref(in0, in1, c0, c1, c2):
        h = (c1 * (in0 + c0).astype(np.float32) ** 2 + c2).astype(np.float32)
        return h ** 16

    spec = Spec(body=body, reference=_ref)
    row = 17
    _SUB_OPCODE_FOR_NAME[name] = row
    tmp = DveOpSpec(name=name, opcode=row, uops=lower(spec, ver="v3"),
                    rd1_en=_has_src1(spec))
    op = DveOp(name, spec, subdim=False,
               uops_sha={"v3": tmp.sha("v3"), "v4": "unpinned"})
    OPS.append(op)
    CUSTOM_DVE_SPECS[name] = spec
    return op


EXP_OP = _register_exp_op()

# kt tiles whose exp runs on the DVE custom op (rest go to ScalarE), strict
# even/odd alternation so the two engines drain the score psum banks in
# lockstep and adjacent score pairs can stay clumped (row-tile concurrency).
DVE_KT = frozenset((1, 3, 5, 7, 9, 11, 13, 15))


def _emit(nc, tc, xT_d, wk_d, wq0_d, wqr_d, wv_d, wo_d, y_d):
    import contextlib
    with contextlib.ExitStack() as ctx:
        persist = ctx.enter_context(tc.tile_pool(name="persist", bufs=1))
        work = ctx.enter_context(tc.tile_pool(name="work", bufs=2))
        spsum = ctx.enter_context(tc.tile_pool(name="spsum", bufs=2, space="PSUM"))
        apsum = ctx.enter_context(tc.tile_pool(name="apsum", bufs=2, space="PSUM"))
        ppsum = ctx.enter_context(tc.tile_pool(name="ppsum", bufs=2, space="PSUM"))

        # ---- static loads ----
        # Consolidated input DMAs: one large contiguous transfer per logical
        # input (host pre-packs so both DRAM and SBUF sides are contiguous =
        # 2-8KB per-partition lines; >=1MB transfers run near peak BW).
        # Need-order: wk + wq pair-0 slice first, then xT in query-column
        # quarters (the upfront K0/Q0 chunks chase quarter arrival), wv before
        # the V weave; wq pairs 1-3 and wo trigger after the upfront chains.
        # SBUF layouts:
        #   xT_all [128, (qc, i, 512)]   wk/wq_all [128, (pair, i, 128)]
        #   wv_all [128, (i, 512)]       wo_all [128, (pair, 1024)]
        xT_all = persist.tile([128, 4 * 4096], BF16, tag="xTall", name="xTall")
        wk_all = persist.tile([128, 4096], BF16, tag="wkall", name="wkall")
        wq_all = persist.tile([128, 4096], BF16, tag="wqall", name="wqall")
        wv_all = persist.tile([128, 4096], BF16, tag="wvall", name="wvall")
        wo_all = persist.tile([128, 4096], BF16, tag="woall", name="woall")

        nc.sync.dma_start(out=wk_all, in_=wk_d[:, :])
        nc.scalar.dma_start(out=wq_all[:, 0:1024], in_=wq0_d[:, :])
        nc.scalar.dma_start(out=wv_all, in_=wv_d[:, :])
        for qc in range(QC):
            nc.sync.dma_start(out=xT_all[:, qc * 4096:(qc + 1) * 4096],
                              in_=xT_d[qc * 128:(qc + 1) * 128, :])

        def xq(i, qc):
            # xT ctile i, query-column chunk qc  [128, 512]
            o = qc * 4096 + i * 512
            return xT_all[:, o:o + 512]

        def xv(i, tt):
            # xT ctile i, t-chunk tt columns  [128, 128]
            o = (tt // 4) * 4096 + i * 512 + (tt % 4) * 128
            return xT_all[:, o:o + 128]

        def wslice(wname, c, p):
            # w ctile c, head-pair p column slice  [128, 128]
            all_ = wk_all if wname == "wk" else wq_all
            return all_[:, p * 1024 + c * 128:p * 1024 + (c + 1) * 128]

        ebias = persist.tile([128, 1], F32, tag="ebias", name="ebias")
        nc.vector.memset(ebias, EXP_ACT_BIAS)

        # V natural [T, 512] + appended ones column per head:
        # vaug [128, 8, 65] with [:, h, 0:64] = V_h and [:, h, 64] = 1
        vaug = [persist.tile([128, HPC, D + 1], BF16, tag=f"vaug{tt}", name=f"vaug{tt}")
                for tt in range(TT)]

        def v_chunk(tt):
            vt = vaug[tt]
            pool, tg = (ppsum, "proj") if tt % 2 == 0 else (apsum, "acc")
            ps = pool.tile([128, 512], F32, tag=tg, name="vps")
            for c in range(CT):
                nc.tensor.matmul(ps, lhsT=xv(c, tt),
                                 rhs=wv_all[:, c * 512:(c + 1) * 512],
                                 start=(c == 0), stop=(c == CT - 1))
            nc.vector.tensor_copy(
                out=vt[:, :, 0:D],
                in_=ps.rearrange("p (h d) -> p h d", h=HPC))
            nc.vector.memset(vt[:, :, D:D + 1], 1.0)

        # Q^T / K^T tiles [128 = 2 heads x 64, T]; filled lazily per pair so
        # later pairs' projections overlap earlier pairs' attention
        qt_sb = [persist.tile([128, T], BF16, tag="qt", bufs=2, name=f"qt{p}")
                 for p in range(PAIRS)]
        kt_sb = [persist.tile([128, T], BF16, tag="kt", bufs=2, name=f"kt{p}")
                 for p in range(PAIRS)]

        def project_chunk(p, dst, wname, qc):
            ps = ppsum.tile([128, 512], F32, tag="proj", name="qkps")
            for c in range(CT):
                nc.tensor.matmul(
                    ps,
                    lhsT=wslice(wname, c, p),
                    rhs=xq(c, qc),
                    start=(c == 0), stop=(c == CT - 1))
            # evacuate on ACT ([128,512] copy is cheaper there and DVE is
            # loaded with the exp split)
            nc.scalar.copy(out=dst[:, qc * 512:(qc + 1) * 512], in_=ps)

        # pair-0 K/Q upfront, interleaved per query chunk so the chains chase
        # the qc-major xT quarter arrival (the DMA-wait window is free PE
        # time); scores (0,0) need all of K0 so sections start right after.
        project_chunk(0, kt_sb[0], "wk", 0)
        project_chunk(0, qt_sb[0], "wq", 0)
        # V chunks 0-3 need only wv + the first xT quarter: they fill the
        # PE idle while the remaining quarters stream in
        for tt in range(4):
            v_chunk(tt)
        for qc in range(1, QC):
            project_chunk(0, kt_sb[0], "wk", qc)
            project_chunk(0, qt_sb[0], "wq", qc)
        # non-critical weights trail the xT quarters in the scalar queue
        nc.scalar.dma_start(out=wq_all[:, 1024:4096], in_=wqr_d[:, :])
        nc.scalar.dma_start(out=wo_all, in_=wo_d[:, :])

        # ---- attention ----
        # expS ring: interleaved [h0 kt | h1 kt] units of 512, RING units so
        # exp of section s+1 can run ahead while PV of section s drains.
        RING = 56
        exps = persist.tile([128, RING * 512], BF16, tag="expS", name="expS")
        tht_sb = [persist.tile([128, T], BF16, tag=f"tht{p}", name=f"tht{p}")
                  for p in range(PAIRS)]
        # just-in-time projection fillers emitted after each (p, qc) section
        fillers = {
            (0, 1): [(1, "wk", 0), (1, "wk", 1), (1, "wk", 2)],
            (0, 2): [(1, "wk", 3), (1, "wq", 0), (1, "wq", 1)],
            (0, 3): [(1, "wq", 2), (1, "wq", 3)],
            (1, 0): [(2, "wk", 0), (2, "wk", 1)],
            (1, 1): [(2, "wk", 2), (2, "wk", 3)],
            (1, 2): [(2, "wq", 0), (2, "wq", 1)],
            (1, 3): [(2, "wq", 2), (2, "wq", 3)],
            (2, 0): [(3, "wk", 0), (3, "wk", 1)],
            (2, 1): [(3, "wk", 2), (3, "wk", 3)],
            (2, 2): [(3, "wq", 0), (3, "wq", 1)],
            (2, 3): [(3, "wq", 2), (3, "wq", 3)],
        }

        def out_proj_group(tt):
            # per-jc copy + DMA so the first half drains while the second
            # half's accumulation is still running
            ysb = work.tile([128, C], BF16, tag="ysb", bufs=3, name="ysb")
            for jc in range(JC):
                jsl = slice(jc * 512, (jc + 1) * 512)
                # both halves on the ppsum ring: keeps the apsum ring free
                # for the PV accumulators, so PV start never waits on a
                # group evacuation copy
                yps = ppsum.tile([128, 512], F32, tag="proj", name="yps")
                for pp in range(PAIRS):
                    nc.tensor.matmul(
                        yps, lhsT=tht_sb[pp][:, tt * 128:(tt + 1) * 128],
                        rhs=wo_all[:, pp * 1024 + jc * 512:
                                   pp * 1024 + (jc + 1) * 512],
                        start=(pp == 0), stop=(pp == PAIRS - 1))
                # evacuation split across ScalarE/VectorE so neither paces
                # the out-proj chains
                if jc == 0:
                    nc.scalar.copy(out=ysb[:, jsl], in_=yps)
                else:
                    nc.vector.tensor_copy(out=ysb[:, jsl], in_=yps)
                eng = nc.sync if (tt + jc) % 2 == 0 else nc.gpsimd
                eng.dma_start(out=y_d[tt * 128:(tt + 1) * 128, jsl],
                              in_=ysb[:, jsl])

        ring_base = 0
        for p in range(PAIRS):
            for qc in range(QC):
                qsl = slice(qc * 512, (qc + 1) * 512)

                def unit(kt, lh):
                    u = (ring_base + 2 * kt + lh) % RING
                    return slice(u * 512, (u + 1) * 512)

                # scores + exp: adjacent matmuls alternate PE row groups
                # (h0 rows 0-63, h1 rows 64-127) so they can overlap
                for kt in range(TT):
                    ps = spsum.tile([128, 1024], F32, tag="mm", name="sps")
                    with tc.high_priority():
                        for lh in range(2):
                            hsl = slice(lh * 64, (lh + 1) * 64)
                            nc.tensor.matmul(
                                ps[:, lh * 512:(lh + 1) * 512],
                                lhsT=kt_sb[p][hsl, kt * 128:(kt + 1) * 128],
                                rhs=qt_sb[p][hsl, qsl],
                                start=True, stop=True)
                    u0 = (ring_base + 2 * kt) % RING
                    eout = exps[:, u0 * 512:(u0 + 2) * 512]
                    if kt in DVE_KT:
                        nc.vector._custom_dve(EXP_OP, out=eout, in0=ps,
                                              s0=EXP_C0, s1=EXP_C1,
                                              imm2=EXP_C2)
                    else:
                        nc.scalar.activation(out=eout, in_=ps, func=EXP,
                                             scale=0.125, bias=ebias)
                    if p == 0 and qc == 0 and kt >= 4:
                        # V projection woven into the exp-paced score loop
                        # (chunks 0-3 were projected in the prologue)
                        v_chunk(kt)
                # out-projection of the previous qc chunk, placed between
                # scores and PV
                if p == PAIRS - 1 and qc >= 1:
                    for tt in range(4 * (qc - 1), 4 * qc):
                        out_proj_group(tt)

                # PV: both heads' accumulation chains interleaved; row 0 of
                # each chain is the softmax denominator (ones-first vaug)
                ops = [apsum.tile([D + 1, 512], F32, tag="acc", name=f"ops{lh}")
                       for lh in range(2)]
                for kt in range(TT):
                    for lh in range(2):
                        nc.tensor.matmul(
                            ops[lh], lhsT=vaug[kt][:, 2 * p + lh, :],
                            rhs=exps[:, unit(kt, lh)],
                            start=(kt == 0), stop=(kt == TT - 1))
                for lh in range(2):
                    # move the sums to partition 0 on ACT (the custom-DVE
                    # fast reciprocal misreads partition-shifted inputs, and
                    # PSUM partition starts must be 32-aligned anyway)
                    ssb = work.tile([1, 512], F32, tag="ssb", name="ssb")
                    nc.scalar.copy(out=ssb, in_=ops[lh][D:D + 1, :])
                    rsb = work.tile([1, 512], F32, tag="rsb", name="rsb")
                    nc.vector.reciprocal_approx_fast(out=rsb, in_=ssb)
                    rbc = work.tile([64, 512], F32, tag="rbc", name="rbc")
                    nc.gpsimd.partition_broadcast(rbc, rsb)
                    nc.vector.tensor_mul(
                        out=tht_sb[p][lh * 64:(lh + 1) * 64, qsl],
                        in0=ops[lh][0:D, :], in1=rbc)
                ring_base = (ring_base + 2 * TT) % RING
                for fp, wname, fqc in fillers.get((p, qc), []):
                    dst = qt_sb[fp] if wname == "wq" else kt_sb[fp]
                    project_chunk(fp, dst, wname, fqc)

        # ---- tail ----
        # Pre-accumulate pairs 0-2 of out-proj groups 12-14 into borrowed
        # PSUM (scores banks are drained, ppsum has no fillers left). These
        # MMs depend only on earlier sections' theta, so they keep the PE
        # streaming (and HAM warm) while the final normalization chain runs;
        # then each group finishes with just the pair-3 rows. tt15 reuses the
        # spsum ring slot freed by tt12's evacuation.
        pre_yps = {}
        for tt in (12, 13):
            pt = spsum.tile([128, 1024], F32, tag="mm", name=f"pre{tt}")
            pre_yps[tt] = (pt[:, 0:512], pt[:, 512:1024])
        pre_yps[14] = (
            ppsum.tile([128, 512], F32, tag="proj", name="pre14a"),
            ppsum.tile([128, 512], F32, tag="proj", name="pre14b"))
        for tt in (12, 13, 14):
            for jc in range(JC):
                for pp in range(PAIRS - 1):
                    nc.tensor.matmul(
                        pre_yps[tt][jc],
                        lhsT=tht_sb[pp][:, tt * 128:(tt + 1) * 128],
                        rhs=wo_all[:, pp * 1024 + jc * 512:
                                   pp * 1024 + (jc + 1) * 512],
                        start=(pp == 0), stop=False)
        for tt in (12, 13, 14):
            ysb = work.tile([128, C], BF16, tag="ysb", bufs=3, name="ysb")
            for jc in range(JC):
                nc.tensor.matmul(
                    pre_yps[tt][jc],
                    lhsT=tht_sb[3][:, tt * 128:(tt + 1) * 128],
                    rhs=wo_all[:, 3 * 1024 + jc * 512:3 * 1024 + (jc + 1) * 512],
                    start=False, stop=True)
            for jc in range(JC):
                jsl = slice(jc * 512, (jc + 1) * 512)
                if jc == 0:
                    nc.scalar.copy(out=ysb[:, jsl], in_=pre_yps[tt][jc])
                else:
                    nc.vector.tensor_copy(out=ysb[:, jsl], in_=pre_yps[tt][jc])
                eng = nc.sync if (tt + jc) % 2 == 0 else nc.gpsimd
                eng.dma_start(out=y_d[tt * 128:(tt + 1) * 128, jsl],
                              in_=ysb[:, jsl])
        # tt15 runs through the apsum ring: its slots free as soon as the
        # final normalization reads the PV accumulators (~norm time), unlike
        # the spsum slots which would wait for the pre12 evacuation copies
        ysb = work.tile([128, C], BF16, tag="ysb", bufs=3, name="ysb")
        for jc in range(JC):
            jsl = slice(jc * 512, (jc + 1) * 512)
            yps = apsum.tile([128, 512], F32, tag="acc", name="yps15")
            for pp in range(PAIRS):
                nc.tensor.matmul(
                    yps,
                    lhsT=tht_sb[pp][:, 15 * 128:16 * 128],
                    rhs=wo_all[:, pp * 1024 + jc * 512:
                               pp * 1024 + (jc + 1) * 512],
                    start=(pp == 0), stop=(pp == PAIRS - 1))
            if jc == 0:
                nc.scalar.copy(out=ysb[:, jsl], in_=yps)
            else:
                nc.vector.tensor_copy(out=ysb[:, jsl], in_=yps)
            eng = nc.sync if jc == 0 else nc.gpsimd
            eng.dma_start(out=y_d[15 * 128:16 * 128, jsl], in_=ysb[:, jsl])


def _build():
    nc = bacc.Bacc("TRN2", target_bir_lowering=False)
    # host-prepacked layouts (see make_in_maps): xT quarters [qc][p, (i, c)],
    # wk/wq pair-major [p, (pair, i, 128)], wv [p, (i, 512)], wo [p, (pair, C)]
    xT_d = nc.dram_tensor("xT", [512, 4096], BF16, kind="ExternalInput")
    wk_d = nc.dram_tensor("wk", [128, 4096], BF16, kind="ExternalInput")
    wq0_d = nc.dram_tensor("wq0", [128, 1024], BF16, kind="ExternalInput")
    wqr_d = nc.dram_tensor("wqr", [128, 3072], BF16, kind="ExternalInput")
    wv_d = nc.dram_tensor("wv", [128, 4096], BF16, kind="ExternalInput")
    wo_d = nc.dram_tensor("wo", [128, 4096], BF16, kind="ExternalInput")
    y_d = nc.dram_tensor("y", [T, C], BF16, kind="ExternalOutput")
    with tile.TileContext(nc) as tc:
        _emit(nc, tc, xT_d, wk_d, wq0_d, wqr_d, wv_d, wo_d, y_d)
    if not nc.is_finalized():
        nc.finalize()
    return nc


def get_nc():
    global _CACHED_NC
    if _CACHED_NC is None:
        _CACHED_NC = _build()
    return _CACHED_NC


def make_in_maps(x, w_qkv, w_out, b_out):
    bf = ml_dtypes.bfloat16
    x = np.asarray(x, dtype=np.float32)
    w_qkv = np.asarray(w_qkv, dtype=np.float32)
    w_out = np.asarray(w_out, dtype=np.float32)
    in_maps = []
    for core in range(8):
        b, hg = core // 2, core % 2
        cs = slice(hg * 512, (hg + 1) * 512)
        xT = x[b].T  # [C, T]
        wq = w_qkv[:, 0 * C:][:, cs]  # [C, 512]
        wk = w_qkv[:, 1 * C:][:, cs]
        wv = w_qkv[:, 2 * C:][:, cs]
        wo = w_out[cs, :]  # [512, C]
        # pack so each DMA is contiguous on both DRAM and SBUF sides:
        #   xT   -> [qc, p, (i, 512)]    (query-column quarters)
        #   wk   -> [p, (pair, i, 128)]  (head-pair-major column slices)
        #   wq   -> pair-0 block [p, (i, 128)] + pairs 1-3 [p, (pair, i, 128)]
        #   wv   -> [p, (i, 512)]
        #   wo   -> [p, (pair, C)]
        xT4 = xT.reshape(8, 128, 4, 512).transpose(2, 1, 0, 3).reshape(512, 4096)
        wk4 = wk.reshape(8, 128, 4, 128).transpose(1, 2, 0, 3).reshape(128, 4096)
        wq4 = wq.reshape(8, 128, 4, 128)
        wq0 = wq4[:, :, 0, :].transpose(1, 0, 2).reshape(128, 1024)
        wqr = wq4[:, :, 1:, :].transpose(1, 2, 0, 3).reshape(128, 3072)
        wv4 = wv.reshape(8, 128, 512).transpose(1, 0, 2).reshape(128, 4096)
        wo4 = wo.reshape(4, 128, 1024).transpose(1, 0, 2).reshape(128, 4096)
        in_maps.append({
            "xT": np.ascontiguousarray(xT4).astype(bf),
            "wk": np.ascontiguousarray(wk4).astype(bf),
            "wq0": np.ascontiguousarray(wq0).astype(bf),
            "wqr": np.ascontiguousarray(wqr).astype(bf),
            "wv": np.ascontiguousarray(wv4).astype(bf),
            "wo": np.ascontiguousarray(wo4).astype(bf),
        })
    return in_maps


def _ensure_ntff_hook():
    """Register the axon NTFF profile hook if the container's antenv lacks
    axon_hooks (test/profiling use only; never needed for plain kernel())."""
    import sys
    import types
    try:
        from antenv import axon_hooks  # noqa: F401
    except ImportError:
        mod = types.ModuleType("antenv.axon_hooks")
        mod._hook = None

        def set_axon_ntff_profile_hook(hook, _m=mod):
            _m._hook = hook

        def get_axon_ntff_profile_hook(_m=mod):
            return _m._hook

        mod.set_axon_ntff_profile_hook = set_axon_ntff_profile_hook
        mod.get_axon_ntff_profile_hook = get_axon_ntff_profile_hook
        sys.modules["antenv.axon_hooks"] = mod
        import antenv
        antenv.axon_hooks = mod
    import antenv.axon_hooks as ah
    if ah.get_axon_ntff_profile_hook() is None:
        from trn_agent_boot.trn_boot import _ntff_profile_via_ctypes
        ah.set_axon_ntff_profile_hook(
            _ntff_profile_via_ctypes("/opt/axon/libaxon_pjrt.so"))


def kernel(x, w_qkv, w_out, b_out, _trace=False, _trace_kwargs=None):
    nc = get_nc()
    in_maps = make_in_maps(x, w_qkv, w_out, b_out)
    kwargs = {}
    if _trace:
        try:
            _ensure_ntff_hook()
        except Exception as e:
            print(f"NTFF hook setup failed ({e}); running without trace")
        else:
            kwargs.update(trace=True, **(_trace_kwargs or {}))
    res = run_bass_kernel_spmd(nc, in_maps, core_ids=list(range(8)), **kwargs)
    bias = np.asarray(b_out, dtype=np.float32)
    out = np.empty((B, T, C), dtype=np.float32)
    for b in range(B):
        out[b] = (res.results[2 * b]["y"].astype(np.float32)
                  + res.results[2 * b + 1]["y"].astype(np.float32) + bias)
    if _trace:
        return out, res
    return out



# revision 37
# speedup vs baseline: 1.1264x; 1.0054x over previous
"""Multi-head attention Trainium2 kernel (B=4, T=2048, C=1024, H=16, D=64).

Sharding: 8 cores = 4 batches x 2 head-groups (data parallel on B, tensor
parallel on H). Each core computes attention for 1 batch and 8 heads plus the
partial out-projection for its head rows; the host sums the two partials per
batch (the out-proj "all-reduce") and adds the bias.

Device layout notes (per core):
  xT  [C, T]   bf16  x[b] transposed + repacked on host into query-column
               quarters so each quarter is one contiguous ~1MB DMA
  wq/wk/wv [C, 512] bf16 per-head-group column slices of w_qkv, repacked
               pair-major (wk/wq) / ctile-major (wv) for single-DMA loads
  wo  [512, C] bf16  row slice of w_out, pair-major
  y   [T, C]   bf16  partial output (host sums the two hg partials in f32)

  Inputs stream in first-use order (wk, wv, wq pair-0, xT quarters; wq
  pairs 1-3 + wo trail after the upfront chains); the upfront K0/Q0
  projection chunks and the first four V chunks chase the quarter arrival
  so the PE has work during the DMA-bound window. The final out-proj row
  groups are pre-accumulated (pairs 0-2) while the last softmax
  normalization chain runs, so the tail only adds the pair-3 rows.

  QT/KT: [D,T] per head, two heads packed per 128-partition tile. Scores
  S^T[k,q] matmuls alternate the two heads (disjoint PE row groups) so
  consecutive matmuls can overlap in the array. The exp of the scores is
  split between ScalarE (exact exp, scaled by K) and a custom 8-stage DVE
  ucode op computing K*exp(x/8) ~= (c1*(x+c0)^2+c2)^16 (error <0.2% where
  softmax mass lives); the constant K cancels in the softmax ratio. expS
  lands in an interleaved bf16 ring in SBUF. V is kept natural [T,D] with a
  appended ones column so the M=65 PV matmul produces O^T (rows 0..63) and
  the softmax denominators (row 64) in one pass. Denominators are copied to
  partition 0 by ScalarE, reciprocal via fast DVE approx, partition
  broadcast on GpSimd, normalization mul on DVE, then the
  out-projection consumes Theta^T as the stationary operand; y is copied to
  bf16 by ScalarE and DMA'd out.
"""

import numpy as np
import ml_dtypes

import concourse.bacc as bacc
import concourse.mybir as mybir
import concourse.tile as tile
from concourse.bass_utils import run_bass_kernel_spmd

B, T, C, H, D = 4, 2048, 1024, 16, 64
HPC = 8          # heads per core
PAIRS = HPC // 2
CT = C // 128    # 8 contraction tiles for projections
TT = T // 128    # 16 t-tiles (also k-tiles of attention)
QC = T // 512    # 4 query chunks
JC = C // 512    # 2 out-proj column chunks
BF16 = mybir.dt.bfloat16
F32 = mybir.dt.float32
EXP = mybir.ActivationFunctionType.Exp

_CACHED_NC = None

# ---- custom DVE exp op: K*exp(s/8) ~= (C1*(s+C0)^2 + C2)^16 --------------
# Constants fitted to minimize attention-output error for logits ~ N(0,1)
# mixed half/half with the exact-exp (ScalarE) path; K = 0.20595367 is the
# shared scale (cancels in softmax), applied on the ACT side via bias=ln K.
EXP_C0 = 113.8532448      # 8*a: input is raw scores s, logit = s/8
EXP_C1 = 3.104248719e-05  # c/64
EXP_C2 = 0.504467297
EXP_ACT_BIAS = -1.5801040383996299  # ln(0.20595367)


def _register_exp_op():
    from concourse.dve_ops import (DveOp, OPS, CUSTOM_DVE_SPECS,
                                   _SUB_OPCODE_FOR_NAME)
    from concourse.dve_spec import Spec, Src0, C0, C1, C2, lower, _has_src1
    from concourse.dve_uop import DveOpSpec

    name = "EXP_POW16_ANT"
    if name in _SUB_OPCODE_FOR_NAME:
        return next(o for o in OPS if o.name == name)
    _t = Src0 + C0
    _s = _t * _t
    _u = _s * C1
    _h = _u + C2
    _h2 = _h * _h
    _h4 = _h2 * _h2
    _h8 = _h4 * _h4
    body = _h8 * _h8

    def _ref(in0, in1, c0, c1, c2):
        h = (c1 * (in0 + c0).astype(np.float32) ** 2 + c2).astype(np.float32)
        return h ** 16

    spec = Spec(body=body, reference=_ref)
    row = 17
    _SUB_OPCODE_FOR_NAME[name] = row
    tmp = DveOpSpec(name=name, opcode=row, uops=lower(spec, ver="v3"),
                    rd1_en=_has_src1(spec))
    op = DveOp(name, spec, subdim=False,
               uops_sha={"v3": tmp.sha("v3"), "v4": "unpinned"})
    OPS.append(op)
    CUSTOM_DVE_SPECS[name] = spec
    return op


EXP_OP = _register_exp_op()

# kt tiles whose exp runs on the DVE custom op (rest go to ScalarE), strict
# even/odd alternation so the two engines drain the score psum banks in
# lockstep and adjacent score pairs can stay clumped (row-tile concurrency).
DVE_KT = frozenset((1, 3, 5, 7, 9, 11, 13, 15))


def _emit(nc, tc, xT_d, wk0_d, wkr_d, wq0_d, wqr_d, wv_d, wo_d, y_d):
    import contextlib
    with contextlib.ExitStack() as ctx:
        persist = ctx.enter_context(tc.tile_pool(name="persist", bufs=1))
        work = ctx.enter_context(tc.tile_pool(name="work", bufs=2))
        spsum = ctx.enter_context(tc.tile_pool(name="spsum", bufs=2, space="PSUM"))
        apsum = ctx.enter_context(tc.tile_pool(name="apsum", bufs=2, space="PSUM"))
        ppsum = ctx.enter_context(tc.tile_pool(name="ppsum", bufs=2, space="PSUM"))

        # ---- static loads ----
        # Consolidated input DMAs: one large contiguous transfer per logical
        # input (host pre-packs so both DRAM and SBUF sides are contiguous =
        # 2-8KB per-partition lines; >=1MB transfers run near peak BW).
        # Need-order: wk + wq pair-0 slice first, then xT in query-column
        # quarters (the upfront K0/Q0 chunks chase quarter arrival), wv before
        # the V weave; wq pairs 1-3 and wo trigger after the upfront chains.
        # SBUF layouts:
        #   xT_all [128, (qc, i, 512)]   wk/wq_all [128, (pair, i, 128)]
        #   wv_all [128, (i, 512)]       wo_all [128, (pair, 1024)]
        xT_all = persist.tile([128, 4 * 4096], BF16, tag="xTall", name="xTall")
        wk_all = persist.tile([128, 4096], BF16, tag="wkall", name="wkall")
        wq_all = persist.tile([128, 4096], BF16, tag="wqall", name="wqall")
        wv_all = persist.tile([128, 4096], BF16, tag="wvall", name="wvall")
        wo_all = persist.tile([128, 4096], BF16, tag="woall", name="woall")

        nc.sync.dma_start(out=wk_all[:, 0:1024], in_=wk0_d[:, :])
        nc.scalar.dma_start(out=wq_all[:, 0:1024], in_=wq0_d[:, :])
        nc.scalar.dma_start(out=wv_all, in_=wv_d[:, :])
        for qc in range(QC):
            nc.sync.dma_start(out=xT_all[:, qc * 4096:(qc + 1) * 4096],
                              in_=xT_d[qc * 128:(qc + 1) * 128, :])

        def xq(i, qc):
            # xT ctile i, query-column chunk qc  [128, 512]
            o = qc * 4096 + i * 512
            return xT_all[:, o:o + 512]

        def xv(i, tt):
            # xT ctile i, t-chunk tt columns  [128, 128]
            o = (tt // 4) * 4096 + i * 512 + (tt % 4) * 128
            return xT_all[:, o:o + 128]

        def wslice(wname, c, p):
            # w ctile c, head-pair p column slice  [128, 128]
            all_ = wk_all if wname == "wk" else wq_all
            return all_[:, p * 1024 + c * 128:p * 1024 + (c + 1) * 128]

        ebias = persist.tile([128, 1], F32, tag="ebias", name="ebias")
        nc.vector.memset(ebias, EXP_ACT_BIAS)

        # V natural [T, 512] + appended ones column per head:
        # vaug [128, 8, 65] with [:, h, 0:64] = V_h and [:, h, 64] = 1
        vaug = [persist.tile([128, HPC, D + 1], BF16, tag=f"vaug{tt}", name=f"vaug{tt}")
                for tt in range(TT)]

        def v_chunk(tt):
            vt = vaug[tt]
            pool, tg = (ppsum, "proj") if tt % 2 == 0 else (apsum, "acc")
            ps = pool.tile([128, 512], F32, tag=tg, name="vps")
            for c in range(CT):
                nc.tensor.matmul(ps, lhsT=xv(c, tt),
                                 rhs=wv_all[:, c * 512:(c + 1) * 512],
                                 start=(c == 0), stop=(c == CT - 1))
            nc.vector.tensor_copy(
                out=vt[:, :, 0:D],
                in_=ps.rearrange("p (h d) -> p h d", h=HPC))
            nc.vector.memset(vt[:, :, D:D + 1], 1.0)

        # Q^T / K^T tiles [128 = 2 heads x 64, T]; filled lazily per pair so
        # later pairs' projections overlap earlier pairs' attention
        qt_sb = [persist.tile([128, T], BF16, tag="qt", bufs=2, name=f"qt{p}")
                 for p in range(PAIRS)]
        kt_sb = [persist.tile([128, T], BF16, tag="kt", bufs=2, name=f"kt{p}")
                 for p in range(PAIRS)]

        def project_chunk(p, dst, wname, qc):
            ps = ppsum.tile([128, 512], F32, tag="proj", name="qkps")
            for c in range(CT):
                nc.tensor.matmul(
                    ps,
                    lhsT=wslice(wname, c, p),
                    rhs=xq(c, qc),
                    start=(c == 0), stop=(c == CT - 1))
            # evacuate on ACT ([128,512] copy is cheaper there and DVE is
            # loaded with the exp split)
            nc.scalar.copy(out=dst[:, qc * 512:(qc + 1) * 512], in_=ps)

        # pair-0 K/Q upfront, interleaved per query chunk so the chains chase
        # the qc-major xT quarter arrival (the DMA-wait window is free PE
        # time); scores (0,0) need all of K0 so sections start right after.
        project_chunk(0, kt_sb[0], "wk", 0)
        project_chunk(0, qt_sb[0], "wq", 0)
        # V chunks 0-3 need only wv + the first xT quarter: they fill the
        # PE idle while the remaining quarters stream in
        for tt in range(4):
            v_chunk(tt)
        for qc in range(1, QC):
            project_chunk(0, kt_sb[0], "wk", qc)
            project_chunk(0, qt_sb[0], "wq", qc)
        # non-critical weights trail the xT quarters in the scalar queue
        nc.scalar.dma_start(out=wk_all[:, 1024:4096], in_=wkr_d[:, :])
        nc.scalar.dma_start(out=wq_all[:, 1024:4096], in_=wqr_d[:, :])
        nc.scalar.dma_start(out=wo_all, in_=wo_d[:, :])

        # ---- attention ----
        # expS ring: interleaved [h0 kt | h1 kt] units of 512, RING units so
        # exp of section s+1 can run ahead while PV of section s drains.
        RING = 56
        exps = persist.tile([128, RING * 512], BF16, tag="expS", name="expS")
        tht_sb = [persist.tile([128, T], BF16, tag=f"tht{p}", name=f"tht{p}")
                  for p in range(PAIRS)]
        # just-in-time projection fillers emitted after each (p, qc) section
        fillers = {
            (0, 1): [(1, "wk", 0), (1, "wk", 1), (1, "wk", 2)],
            (0, 2): [(1, "wk", 3), (1, "wq", 0), (1, "wq", 1)],
            (0, 3): [(1, "wq", 2), (1, "wq", 3)],
            (1, 0): [(2, "wk", 0), (2, "wk", 1)],
            (1, 1): [(2, "wk", 2), (2, "wk", 3)],
            (1, 2): [(2, "wq", 0), (2, "wq", 1)],
            (1, 3): [(2, "wq", 2), (2, "wq", 3)],
            (2, 0): [(3, "wk", 0), (3, "wk", 1)],
            (2, 1): [(3, "wk", 2), (3, "wk", 3)],
            (2, 2): [(3, "wq", 0), (3, "wq", 1)],
            (2, 3): [(3, "wq", 2), (3, "wq", 3)],
        }

        def out_proj_group(tt):
            # per-jc copy + DMA so the first half drains while the second
            # half's accumulation is still running
            ysb = work.tile([128, C], BF16, tag="ysb", bufs=3, name="ysb")
            for jc in range(JC):
                jsl = slice(jc * 512, (jc + 1) * 512)
                # both halves on the ppsum ring: keeps the apsum ring free
                # for the PV accumulators, so PV start never waits on a
                # group evacuation copy
                yps = ppsum.tile([128, 512], F32, tag="proj", name="yps")
                for pp in range(PAIRS):
                    nc.tensor.matmul(
                        yps, lhsT=tht_sb[pp][:, tt * 128:(tt + 1) * 128],
                        rhs=wo_all[:, pp * 1024 + jc * 512:
                                   pp * 1024 + (jc + 1) * 512],
                        start=(pp == 0), stop=(pp == PAIRS - 1))
                # evacuation split across ScalarE/VectorE so neither paces
                # the out-proj chains
                if jc == 0:
                    nc.scalar.copy(out=ysb[:, jsl], in_=yps)
                else:
                    nc.vector.tensor_copy(out=ysb[:, jsl], in_=yps)
                eng = nc.sync if (tt + jc) % 2 == 0 else nc.gpsimd
                eng.dma_start(out=y_d[tt * 128:(tt + 1) * 128, jsl],
                              in_=ysb[:, jsl])

        ring_base = 0
        for p in range(PAIRS):
            for qc in range(QC):
                qsl = slice(qc * 512, (qc + 1) * 512)

                def unit(kt, lh):
                    u = (ring_base + 2 * kt + lh) % RING
                    return slice(u * 512, (u + 1) * 512)

                # scores + exp: adjacent matmuls alternate PE row groups
                # (h0 rows 0-63, h1 rows 64-127) so they can overlap
                for kt in range(TT):
                    ps = spsum.tile([128, 1024], F32, tag="mm", name="sps")
                    with tc.high_priority():
                        for lh in range(2):
                            hsl = slice(lh * 64, (lh + 1) * 64)
                            nc.tensor.matmul(
                                ps[:, lh * 512:(lh + 1) * 512],
                                lhsT=kt_sb[p][hsl, kt * 128:(kt + 1) * 128],
                                rhs=qt_sb[p][hsl, qsl],
                                start=True, stop=True)
                    u0 = (ring_base + 2 * kt) % RING
                    eout = exps[:, u0 * 512:(u0 + 2) * 512]
                    if kt in DVE_KT:
                        nc.vector._custom_dve(EXP_OP, out=eout, in0=ps,
                                              s0=EXP_C0, s1=EXP_C1,
                                              imm2=EXP_C2)
                    else:
                        nc.scalar.activation(out=eout, in_=ps, func=EXP,
                                             scale=0.125, bias=ebias)
                    if p == 0 and qc == 0 and kt >= 4:
                        # V projection woven into the exp-paced score loop
                        # (chunks 0-3 were projected in the prologue)
                        v_chunk(kt)
                # out-projection of the previous qc chunk, placed between
                # scores and PV
                if p == PAIRS - 1 and qc >= 1:
                    for tt in range(4 * (qc - 1), 4 * qc):
                        out_proj_group(tt)

                # PV: both heads' accumulation chains interleaved; row 0 of
                # each chain is the softmax denominator (ones-first vaug)
                ops = [apsum.tile([D + 1, 512], F32, tag="acc", name=f"ops{lh}")
                       for lh in range(2)]
                for kt in range(TT):
                    for lh in range(2):
                        nc.tensor.matmul(
                            ops[lh], lhsT=vaug[kt][:, 2 * p + lh, :],
                            rhs=exps[:, unit(kt, lh)],
                            start=(kt == 0), stop=(kt == TT - 1))
                for lh in range(2):
                    # move the sums to partition 0 on ACT (the custom-DVE
                    # fast reciprocal misreads partition-shifted inputs, and
                    # PSUM partition starts must be 32-aligned anyway)
                    ssb = work.tile([1, 512], F32, tag="ssb", name="ssb")
                    nc.scalar.copy(out=ssb, in_=ops[lh][D:D + 1, :])
                    rsb = work.tile([1, 512], F32, tag="rsb", name="rsb")
                    nc.vector.reciprocal_approx_fast(out=rsb, in_=ssb)
                    rbc = work.tile([64, 512], F32, tag="rbc", name="rbc")
                    nc.gpsimd.partition_broadcast(rbc, rsb)
                    nc.vector.tensor_mul(
                        out=tht_sb[p][lh * 64:(lh + 1) * 64, qsl],
                        in0=ops[lh][0:D, :], in1=rbc)
                ring_base = (ring_base + 2 * TT) % RING
                for fp, wname, fqc in fillers.get((p, qc), []):
                    dst = qt_sb[fp] if wname == "wq" else kt_sb[fp]
                    project_chunk(fp, dst, wname, fqc)

        # ---- tail ----
        # Pre-accumulate pairs 0-2 of out-proj groups 12-14 into borrowed
        # PSUM (scores banks are drained, ppsum has no fillers left). These
        # MMs depend only on earlier sections' theta, so they keep the PE
        # streaming (and HAM warm) while the final normalization chain runs;
        # then each group finishes with just the pair-3 rows. tt15 reuses the
        # spsum ring slot freed by tt12's evacuation.
        pre_yps = {}
        for tt in (12, 13):
            pt = spsum.tile([128, 1024], F32, tag="mm", name=f"pre{tt}")
            pre_yps[tt] = (pt[:, 0:512], pt[:, 512:1024])
        pre_yps[14] = (
            ppsum.tile([128, 512], F32, tag="proj", name="pre14a"),
            ppsum.tile([128, 512], F32, tag="proj", name="pre14b"))
        for tt in (12, 13, 14):
            for jc in range(JC):
                for pp in range(PAIRS - 1):
                    nc.tensor.matmul(
                        pre_yps[tt][jc],
                        lhsT=tht_sb[pp][:, tt * 128:(tt + 1) * 128],
                        rhs=wo_all[:, pp * 1024 + jc * 512:
                                   pp * 1024 + (jc + 1) * 512],
                        start=(pp == 0), stop=False)
        for tt in (12, 13, 14):
            ysb = work.tile([128, C], BF16, tag="ysb", bufs=3, name="ysb")
            for jc in range(JC):
                nc.tensor.matmul(
                    pre_yps[tt][jc],
                    lhsT=tht_sb[3][:, tt * 128:(tt + 1) * 128],
                    rhs=wo_all[:, 3 * 1024 + jc * 512:3 * 1024 + (jc + 1) * 512],
                    start=False, stop=True)
            for jc in range(JC):
                jsl = slice(jc * 512, (jc + 1) * 512)
                if jc == 0:
                    nc.scalar.copy(out=ysb[:, jsl], in_=pre_yps[tt][jc])
                else:
                    nc.vector.tensor_copy(out=ysb[:, jsl], in_=pre_yps[tt][jc])
                eng = nc.sync if (tt + jc) % 2 == 0 else nc.gpsimd
                eng.dma_start(out=y_d[tt * 128:(tt + 1) * 128, jsl],
                              in_=ysb[:, jsl])
        # tt15 runs through the apsum ring: its slots free as soon as the
        # final normalization reads the PV accumulators (~norm time), unlike
        # the spsum slots which would wait for the pre12 evacuation copies
        ysb = work.tile([128, C], BF16, tag="ysb", bufs=3, name="ysb")
        for jc in range(JC):
            jsl = slice(jc * 512, (jc + 1) * 512)
            yps = apsum.tile([128, 512], F32, tag="acc", name="yps15")
            for pp in range(PAIRS):
                nc.tensor.matmul(
                    yps,
                    lhsT=tht_sb[pp][:, 15 * 128:16 * 128],
                    rhs=wo_all[:, pp * 1024 + jc * 512:
                               pp * 1024 + (jc + 1) * 512],
                    start=(pp == 0), stop=(pp == PAIRS - 1))
            if jc == 0:
                nc.scalar.copy(out=ysb[:, jsl], in_=yps)
            else:
                nc.vector.tensor_copy(out=ysb[:, jsl], in_=yps)
            eng = nc.sync if jc == 0 else nc.gpsimd
            eng.dma_start(out=y_d[15 * 128:16 * 128, jsl], in_=ysb[:, jsl])


def _build():
    nc = bacc.Bacc("TRN2", target_bir_lowering=False)
    # host-prepacked layouts (see make_in_maps): xT quarters [qc][p, (i, c)],
    # wk/wq pair-major [p, (pair, i, 128)], wv [p, (i, 512)], wo [p, (pair, C)]
    xT_d = nc.dram_tensor("xT", [512, 4096], BF16, kind="ExternalInput")
    wk0_d = nc.dram_tensor("wk0", [128, 1024], BF16, kind="ExternalInput")
    wkr_d = nc.dram_tensor("wkr", [128, 3072], BF16, kind="ExternalInput")
    wq0_d = nc.dram_tensor("wq0", [128, 1024], BF16, kind="ExternalInput")
    wqr_d = nc.dram_tensor("wqr", [128, 3072], BF16, kind="ExternalInput")
    wv_d = nc.dram_tensor("wv", [128, 4096], BF16, kind="ExternalInput")
    wo_d = nc.dram_tensor("wo", [128, 4096], BF16, kind="ExternalInput")
    y_d = nc.dram_tensor("y", [T, C], BF16, kind="ExternalOutput")
    with tile.TileContext(nc) as tc:
        _emit(nc, tc, xT_d, wk0_d, wkr_d, wq0_d, wqr_d, wv_d, wo_d, y_d)
    if not nc.is_finalized():
        nc.finalize()
    return nc


def get_nc():
    global _CACHED_NC
    if _CACHED_NC is None:
        _CACHED_NC = _build()
    return _CACHED_NC


def make_in_maps(x, w_qkv, w_out, b_out):
    bf = ml_dtypes.bfloat16
    x = np.asarray(x, dtype=np.float32)
    w_qkv = np.asarray(w_qkv, dtype=np.float32)
    w_out = np.asarray(w_out, dtype=np.float32)
    in_maps = []
    for core in range(8):
        b, hg = core // 2, core % 2
        cs = slice(hg * 512, (hg + 1) * 512)
        xT = x[b].T  # [C, T]
        wq = w_qkv[:, 0 * C:][:, cs]  # [C, 512]
        wk = w_qkv[:, 1 * C:][:, cs]
        wv = w_qkv[:, 2 * C:][:, cs]
        wo = w_out[cs, :]  # [512, C]
        # pack so each DMA is contiguous on both DRAM and SBUF sides:
        #   xT   -> [qc, p, (i, 512)]    (query-column quarters)
        #   wk   -> [p, (pair, i, 128)]  (head-pair-major column slices)
        #   wq   -> pair-0 block [p, (i, 128)] + pairs 1-3 [p, (pair, i, 128)]
        #   wv   -> [p, (i, 512)]
        #   wo   -> [p, (pair, C)]
        xT4 = xT.reshape(8, 128, 4, 512).transpose(2, 1, 0, 3).reshape(512, 4096)
        wk4 = wk.reshape(8, 128, 4, 128)
        wk0 = wk4[:, :, 0, :].transpose(1, 0, 2).reshape(128, 1024)
        wkr = wk4[:, :, 1:, :].transpose(1, 2, 0, 3).reshape(128, 3072)
        wq4 = wq.reshape(8, 128, 4, 128)
        wq0 = wq4[:, :, 0, :].transpose(1, 0, 2).reshape(128, 1024)
        wqr = wq4[:, :, 1:, :].transpose(1, 2, 0, 3).reshape(128, 3072)
        wv4 = wv.reshape(8, 128, 512).transpose(1, 0, 2).reshape(128, 4096)
        wo4 = wo.reshape(4, 128, 1024).transpose(1, 0, 2).reshape(128, 4096)
        in_maps.append({
            "xT": np.ascontiguousarray(xT4).astype(bf),
            "wk0": np.ascontiguousarray(wk0).astype(bf),
            "wkr": np.ascontiguousarray(wkr).astype(bf),
            "wq0": np.ascontiguousarray(wq0).astype(bf),
            "wqr": np.ascontiguousarray(wqr).astype(bf),
            "wv": np.ascontiguousarray(wv4).astype(bf),
            "wo": np.ascontiguousarray(wo4).astype(bf),
        })
    return in_maps


def _ensure_ntff_hook():
    """Register the axon NTFF profile hook if the container's antenv lacks
    axon_hooks (test/profiling use only; never needed for plain kernel())."""
    import sys
    import types
    try:
        from antenv import axon_hooks  # noqa: F401
    except ImportError:
        mod = types.ModuleType("antenv.axon_hooks")
        mod._hook = None

        def set_axon_ntff_profile_hook(hook, _m=mod):
            _m._hook = hook

        def get_axon_ntff_profile_hook(_m=mod):
            return _m._hook

        mod.set_axon_ntff_profile_hook = set_axon_ntff_profile_hook
        mod.get_axon_ntff_profile_hook = get_axon_ntff_profile_hook
        sys.modules["antenv.axon_hooks"] = mod
        import antenv
        antenv.axon_hooks = mod
    import antenv.axon_hooks as ah
    if ah.get_axon_ntff_profile_hook() is None:
        from trn_agent_boot.trn_boot import _ntff_profile_via_ctypes
        ah.set_axon_ntff_profile_hook(
            _ntff_profile_via_ctypes("/opt/axon/libaxon_pjrt.so"))


def kernel(x, w_qkv, w_out, b_out, _trace=False, _trace_kwargs=None):
    nc = get_nc()
    in_maps = make_in_maps(x, w_qkv, w_out, b_out)
    kwargs = {}
    if _trace:
        try:
            _ensure_ntff_hook()
        except Exception as e:
            print(f"NTFF hook setup failed ({e}); running without trace")
        else:
            kwargs.update(trace=True, **(_trace_kwargs or {}))
    res = run_bass_kernel_spmd(nc, in_maps, core_ids=list(range(8)), **kwargs)
    bias = np.asarray(b_out, dtype=np.float32)
    out = np.empty((B, T, C), dtype=np.float32)
    for b in range(B):
        out[b] = (res.results[2 * b]["y"].astype(np.float32)
                  + res.results[2 * b + 1]["y"].astype(np.float32) + bias)
    if _trace:
        return out, res
    return out

